# revision 1
# baseline (speedup 1.0000x reference)
"""Trainium2 Bass kernel for nn_ConformerMHSAV3 (LayerNorm + packed-QKV MHSA,
online/causal + offline/full-context variants, stacked output).

Sharding: 8 cores = 4 batches x 2 head-groups (8 heads each).  Each core
computes LN + its head-group's QKV + attention (both variants) + a partial
output projection; the host sums the two head-group partials per batch and
adds the output bias.

Everything runs in fp32 with fp32r matmuls (full PE rate at N>=256).
Softmax uses a constant shift instead of a row max (scores are O(1) after
LayerNorm + 1/sqrt scaling), with key-padding folded into the exp bias and
the attention mask applied as block-sparse 0/1 multiplies only where the
mask block is mixed (computed from the actual mask at build time, so any
mask pattern is handled; causal gets the fast path).
"""

import numpy as np

import concourse.bass as bass
import concourse.mybir as mybir
import concourse.tile as tile
from concourse import mybir as _mybir
from concourse.bass_utils import run_bass_kernel_spmd
from concourse.vector_clock import ScopedClock, VectorClock

# ---------------------------------------------------------------------------
# Patches for this walrus build's 1-sync-wait-per-instruction cap.
# ---------------------------------------------------------------------------

_MAX_WAITS = 1


def _drain_and_barrier(self, tick_clock, wait_clock):
    gc = ScopedClock({None: tick_clock.global_clock})[None]
    n = len(gc)
    for p in [i for i in range(n) if gc[i] > 0]:
        nop = self.nc.sync.nop(nofuse=True, hint="tail_drain_split")
        partial = VectorClock([gc[j] if j == p else 0 for j in range(n)])
        wait_clock.add_sem_waits(nop.ins, ScopedClock({None: partial}))
    self.nc.sync.drain()
    self.nc.all_engine_barrier()
    assert self.sems is not None
    popped = self.nc._tile_sem_poison_stack.pop()
    assert popped is self._sem_poison
    self.nc.clear_and_free_semaphores(list(self.sems.allocated().values()))
    self.nc.all_engine_barrier()


def _install_patches():
    tile.TileContext._drain_and_barrier = _drain_and_barrier


def _split_multi_waits(nc):
    """Move all-but-one sem wait of each instruction onto same-engine NOPs
    inserted immediately before it (preserves per-engine program order)."""
    for f in nc.m.functions:
        for bb in f.blocks:
            insts = bb.instructions
            i = 0
            while i < len(insts):
                inst = insts[i]
                si = inst.sync_info
                if si is not None and si.on_wait and len(si.on_wait) > _MAX_WAITS:
                    extra = []
                    while len(si.on_wait) > _MAX_WAITS:
                        extra.append(si.on_wait.pop())
                    for w in extra:
                        nop = nc.engines[inst.engine].nop(nofuse=True).ins
                        for blk in f.blocks:
                            if blk.instructions and blk.instructions[-1] is nop:
                                blk.instructions.pop()
                                break
                        if nop.sync_info is None:
                            nop.sync_info = _mybir.SyncInfo(on_wait=[w], on_update=[])
                        else:
                            nop.sync_info.on_wait.append(w)
                        insts.insert(i, nop)
                        i += 1
                i += 1


# ---------------------------------------------------------------------------
# Problem constants (hardcoded per the self-contained-kernel contract).
# ---------------------------------------------------------------------------

B, T, D, H = 4, 1024, 1024, 16
HD = D // H          # 64
HL = H // 2          # 8 local heads per core
P = 128
NT = T // P          # 8 tiles of 128
EPS = 1e-5
C_SHIFT = 12.0       # constant softmax shift (exact-softmax invariant)
NEG = -1e30
F32 = mybir.dt.float32
F32R = mybir.dt.float32r
QB = 256             # online q-block granularity
NQB = T // QB        # 4

_prog_cache = {}


def _classify_blocks(attnT):
    """Per (k-chunk, q-half) classification of the online attention mask at
    PSUM-bank granularity (512 columns), so each bank hosts exactly one
    accumulation group (start=True clears has_written for the whole bank).
    Returns (cls[c][h] in {0:none,1:full,2+idx:masked}, additive blocks)."""
    cls = [[0] * 2 for _ in range(NT)]
    mixed = []
    for c in range(NT):
        for h in range(2):
            blk = attnT[c * P : (c + 1) * P, h * 512 : (h + 1) * 512]
            if blk.all():
                cls[c][h] = 1
            elif not blk.any():
                cls[c][h] = 0
            else:
                cls[c][h] = 2 + len(mixed)
                mixed.append(np.where(blk, 0.0, NEG).astype(np.float32))
    return cls, mixed


def _build_program(used_chunks, cls, n_mixed):
    """Build the SPMD Bass program.  Structure depends only on the masks'
    block classification, which is identical across cores."""
    nc = bass.Bass("TRN2", target_bir_lowering=False, debug=False)

    x_d = nc.declare_dram_parameter("x", [T, D], F32, isOutput=False)
    xT_d = nc.declare_dram_parameter("xT", [D, T], F32, isOutput=False)
    wqkT_d = nc.declare_dram_parameter("wqkT", [D, 2 * HL * HD], F32, isOutput=False)
    wvT_d = nc.declare_dram_parameter("wvT", [D, HL * HD], F32, isOutput=False)
    woT_d = nc.declare_dram_parameter("woT", [HL * HD, D], F32, isOutput=False)
    bqk_d = nc.declare_dram_parameter("bqk", [2 * HL * HD], F32, isOutput=False)
    bv_d = nc.declare_dram_parameter("bv", [HL * HD], F32, isOutput=False)
    seqb_d = nc.declare_dram_parameter("seqb", [P, NT], F32, isOutput=False)
    nmx = max(n_mixed, 1)
    mix_d = nc.declare_dram_parameter("mix", [nmx, P, 512], F32, isOutput=False)
    oon_d = nc.declare_dram_parameter("out_on", [T, D], F32, isOutput=True)
    ooff_d = nc.declare_dram_parameter("out_off", [T, D], F32, isOutput=True)

    ACT = mybir.ActivationFunctionType
    OP = mybir.AluOpType

    first_off, last_off = used_chunks[0], used_chunks[-1]
    first_on = [None, None]
    last_on = [None, None]
    for qh in range(2):
        writers = [c for c in used_chunks if cls[c][qh] != 0]
        if writers:
            first_on[qh], last_on[qh] = writers[0], writers[-1]

    with tile.TileContext(nc) as tc:
        with (
            tc.tile_pool(name="io", bufs=2) as p_io,
            tc.tile_pool(name="big", bufs=2) as p_big,
            tc.tile_pool(name="w1", bufs=1) as p_w1,
            tc.tile_pool(name="w2", bufs=1) as p_w2,
            tc.tile_pool(name="qk", bufs=1) as p_qk,
            tc.tile_pool(name="vv", bufs=1) as p_v,
            tc.tile_pool(name="pp", bufs=4) as p_p,
            tc.tile_pool(name="blk", bufs=2) as p_blk,
            tc.tile_pool(name="bc", bufs=2) as p_bc,
            tc.tile_pool(name="rc", bufs=2) as p_rc,
            tc.tile_pool(name="sm", bufs=1) as p_sm,
            tc.tile_pool(name="st", bufs=3) as p_st,
            tc.tile_pool(name="dram", bufs=2, space="DRAM") as p_dram,
            tc.tile_pool(name="ps", bufs=2, space="PSUM") as p_ps,
            tc.tile_pool(name="po", bufs=2, space="PSUM") as p_po,
        ):
            # ---------------- Phase A: LN statistics --------------------
            mu_all = p_sm.tile([P, NT], F32, tag="mu")
            rstd_all = p_sm.tile([P, NT], F32, tag="rstd")
            mun_all = p_sm.tile([P, NT], F32, tag="mun")
            eps_t = p_sm.tile([P, 1], F32, tag="eps")
            nc.vector.memset(eps_t, EPS)

            for t in range(NT):
                xt = p_io.tile([P, D], F32, tag="io")
                nc.sync.dma_start(out=xt, in_=x_d[t * P : (t + 1) * P, :])
                stats = p_st.tile([P, 2, 6], F32, tag="bst")
                xv = xt.rearrange("p (s q) -> p s q", s=2)
                for s in range(2):
                    nc.vector.bn_stats(out=stats[:, s, :], in_=xv[:, s, :])
                mv = p_st.tile([P, 2], F32, tag="mv")
                nc.vector.bn_aggr(out=mv, in_=stats)
                nc.gpsimd.tensor_copy(out=mu_all[:, t : t + 1], in_=mv[:, 0:1])
                # rstd = 1/sqrt(var + eps)  (same recipe as prod groupnorm)
                nc.scalar.activation(
                    out=rstd_all[:, t : t + 1], in_=mv[:, 1:2],
                    func=ACT.Sqrt, bias=eps_t, scale=1.0,
                )
            nc.vector.reciprocal(out=rstd_all, in_=rstd_all)
            nc.vector.tensor_tensor(out=mun_all, in0=mu_all, in1=rstd_all, op=OP.mult)
            nc.scalar.mul(out=mun_all, in_=mun_all, mul=-1.0)

            # stripe [P, NT] -> DRAM rows (t = 128*tile + p ordering)
            scr = p_dram.tile([2, T], F32, tag="scr")
            nc.sync.dma_start(
                out=scr[0].rearrange("(n p) -> p n", p=P), in_=rstd_all
            )
            nc.sync.dma_start(
                out=scr[1].rearrange("(n p) -> p n", p=P), in_=mun_all
            )
            # broadcast rows across all 128 partitions
            rstd_bc = p_bc.tile([P, T], F32, tag="bc")
            mun_bc = p_bc.tile([P, T], F32, tag="bc")
            row0 = bass.AP(tensor=scr.tensor, offset=scr.offset, ap=[[0, P], [1, T]])
            row1 = bass.AP(tensor=scr.tensor, offset=scr.offset + T, ap=[[0, P], [1, T]])
            nc.sync.dma_start(out=rstd_bc, in_=row0)
            nc.sync.dma_start(out=mun_bc, in_=row1)

            # ---------------- Phase B: normalized transpose -------------
            # xnT[d, t] = xT[d, t] * rstd[t] + (-mu[t]*rstd[t])
            xnT_a = p_big.tile([P, 4, T], F32R, tag="big")
            xnT_b = p_big.tile([P, 4, T], F32R, tag="big")

            def xnT_sl(ko):
                return (xnT_a if ko < 4 else xnT_b)[:, ko % 4, :]

            for ko in range(NT):
                xtt = p_io.tile([P, T], F32, tag="io")
                nc.sync.dma_start(out=xtt, in_=xT_d[ko * P : (ko + 1) * P, :])
                tmp = p_st.tile([P, T], F32, tag="xtmp")
                nc.gpsimd.tensor_tensor(out=tmp, in0=xtt, in1=rstd_bc, op=OP.mult)
                nc.vector.tensor_tensor(out=xnT_sl(ko), in0=tmp, in1=mun_bc, op=OP.add)

            # ---------------- Phase C: qkT = Wqk' @ xn^T ----------------
            wqk_sb = p_w1.tile([P, NT, 2 * HL * HD], F32R, tag="w1")
            nc.sync.dma_start(
                out=wqk_sb, in_=wqkT_d[:].rearrange("(ko p) m -> p ko m", p=P).bitcast(F32R)
            )
            bqk_sb = p_sm.tile([P, NT], F32, tag="bqk")
            nc.sync.dma_start(out=bqk_sb, in_=bqk_d[:].rearrange("(mt p) -> p mt", p=P))
            qkT_sb = p_qk.tile([P, NT, T], F32R, tag="qk")

            for mt in range(NT):
                psq = p_ps.tile([P, T], F32, tag="ps")
                for qh in range(2):
                    for ko in range(NT):
                        nc.tensor.matmul(
                            psq[:, qh * 512 : (qh + 1) * 512],
                            lhsT=wqk_sb[:, ko, mt * P : (mt + 1) * P],
                            rhs=xnT_sl(ko)[:, qh * 512 : (qh + 1) * 512],
                            start=(ko == 0),
                            stop=(ko == NT - 1),
                        )
                nc.scalar.activation(
                    out=qkT_sb[:, mt, :], in_=psq,
                    func=ACT.Identity, bias=bqk_sb[:, mt : mt + 1], scale=1.0,
                )

            # ---------------- Phase D: v (head-interleaved, ones col) ---
            wv_sb = p_w2.tile([P, NT, HL * HD], F32R, tag="w2")
            nc.sync.dma_start(
                out=wv_sb, in_=wvT_d[:].rearrange("(ko p) m -> p ko m", p=P).bitcast(F32R)
            )
            bv_bc = p_st.tile([P, HL * HD], F32, tag="bvbc")
            nc.sync.dma_start(
                out=bv_bc,
                in_=bass.AP(tensor=bv_d, offset=0, ap=[[0, P], [1, HL * HD]]),
            )
            # v_sb[p, c, 65h + j]: j<64 -> v head h dim j; j=64 -> 1.0
            v_sb = p_v.tile([P, NT, HL * 65], F32R, tag="vv")
            nc.vector.memset(
                v_sb.rearrange("p c (h j) -> p c h j", j=65)[:, :, :, 64].bitcast(F32),
                1.0,
            )
            for t in range(NT):
                psv = p_ps.tile([P, T], F32, tag="ps")
                for ko in range(NT):
                    nc.tensor.matmul(
                        psv[:, : HL * HD],
                        lhsT=xnT_sl(ko)[:, t * P : (t + 1) * P],
                        rhs=wv_sb[:, ko, :],
                        start=(ko == 0),
                        stop=(ko == NT - 1),
                    )
                nc.vector.tensor_tensor(
                    out=v_sb.rearrange("p c (h j) -> p c h j", j=65)[:, t, :, 0:64],
                    in0=psv[:, : HL * HD].rearrange("p (h j) -> p h j", j=HD),
                    in1=bv_bc.rearrange("p (h j) -> p h j", j=HD),
                    op=OP.add,
                )

            # ---------------- Phase E: attention per head ---------------
            seqb_sb = p_sm.tile([P, NT], F32, tag="seqb")
            nc.sync.dma_start(out=seqb_sb, in_=seqb_d[:])
            mix_sb = p_w2.tile([P, nmx, 512], F32, tag="mix")
            nc.sync.dma_start(
                out=mix_sb, in_=mix_d[:].rearrange("n p q -> p n q")
            )

            oT_on = p_big.tile([P, 4, T], F32R, tag="big")
            oT_off = p_big.tile([P, 4, T], F32R, tag="big")

            for h in range(HL):
                par = h % 2
                base = 64 * par
                qT_h = qkT_sb[base : base + 64, h // 2, :]
                kT_h = qkT_sb[base : base + 64, 4 + h // 2, :]
                vlo = 65 * h
                pon_t = p_po.tile([P, T], F32, tag="po")
                poff_t = p_po.tile([P, T], F32, tag="po")
                pon = pon_t[0:65]
                poff = poff_t[0:65]

                for ci, c in enumerate(used_chunks):
                    pss = p_ps.tile([P, T], F32, tag="ps")
                    for qh in range(2):
                        nc.tensor.matmul(
                            pss[:, qh * 512 : (qh + 1) * 512],
                            lhsT=kT_h[:, c * P : (c + 1) * P],
                            rhs=qT_h[:, qh * 512 : (qh + 1) * 512],
                            start=True,
                            stop=True,
                        )
                    pofc = p_p.tile([P, T], F32R, tag="pp")
                    nc.scalar.activation(
                        out=pofc, in_=pss, func=ACT.Exp,
                        bias=seqb_sb[:, c : c + 1], scale=1.0,
                    )
                    lhsT = v_sb[:, c, vlo : vlo + 65]
                    for qh in range(2):
                        nc.tensor.matmul(
                            poff[:, qh * 512 : (qh + 1) * 512],
                            lhsT=lhsT,
                            rhs=pofc[:, qh * 512 : (qh + 1) * 512],
                            start=(c == first_off),
                            stop=(c == last_off),
                        )
                    for qh in range(2):
                        k = cls[c][qh]
                        if k == 0 or first_on[qh] is None:
                            continue
                        qsl = slice(qh * 512, (qh + 1) * 512)
                        if k == 1:
                            rhs = pofc[:, qsl]
                        else:
                            nc.vector.tensor_tensor(
                                out=pss[:, qsl],
                                in0=pss[:, qsl],
                                in1=mix_sb[:, k - 2, :],
                                op=OP.add,
                            )
                            pblk = p_blk.tile([P, 512], F32R, tag="blk")
                            nc.scalar.activation(
                                out=pblk, in_=pss[:, qsl], func=ACT.Exp,
                                bias=seqb_sb[:, c : c + 1], scale=1.0,
                            )
                            rhs = pblk
                        nc.tensor.matmul(
                            pon[:, qsl],
                            lhsT=lhsT,
                            rhs=rhs,
                            start=(c == first_on[qh]),
                            stop=(c == last_on[qh]),
                        )

                # divide by the ones-column sums; DVE handles the partition
                # shift to this head's lanes (dlo) directly.
                dlo = base
                for pt, dst in ((pon_t, oT_on), (poff_t, oT_off)):
                    # single ACT copy frees the PSUM slot; the divide chain
                    # then runs from SBUF, overlapped with the next head's PE
                    otmp = p_bc.tile([P, T], F32, tag="bc")
                    nc.scalar.activation(out=otmp[0:65, :], in_=pt[0:65], func=ACT.Copy)
                    drow_dram = p_dram.tile([T], F32, tag="drow")
                    nc.sync.dma_start(out=drow_dram[None, :], in_=otmp[64:65, :])
                    rct = p_rc.tile([P, T], F32, tag="rc")
                    nc.sync.dma_start(
                        out=rct[64:128, :],
                        in_=bass.AP(
                            tensor=drow_dram.tensor,
                            offset=drow_dram.offset,
                            ap=[[0, 64], [1, T]],
                        ),
                    )
                    nc.vector.reciprocal(out=rct[0:64, :], in_=rct[64:128, :])
                    nc.vector.tensor_tensor(
                        out=dst[dlo : dlo + 64, h // 2, :],
                        in0=otmp[0:64],
                        in1=rct[0:64, :],
                        op=OP.mult,
                    )
                # zero any online q-halves no chunk wrote (fully masked)
                for qh in range(2):
                    if first_on[qh] is None:
                        nc.vector.memset(
                            oT_on[dlo : dlo + 64, h // 2, qh * 512 : (qh + 1) * 512]
                            .bitcast(F32),
                            0.0,
                        )

            # ---------------- Phase F: output projection ----------------
            wo_sb = p_w1.tile([P, 4, D], F32R, tag="w1")
            nc.sync.dma_start(
                out=wo_sb, in_=woT_d[:].rearrange("(j p) m -> p j m", p=P).bitcast(F32R)
            )
            for src, dst_d in ((oT_on, oon_d), (oT_off, ooff_d)):
                for t in range(NT):
                    pso = p_ps.tile([P, T], F32, tag="ps")
                    for dh in range(2):
                        for j in range(4):
                            nc.tensor.matmul(
                                pso[:, dh * 512 : (dh + 1) * 512],
                                lhsT=src[:, j, t * P : (t + 1) * P],
                                rhs=wo_sb[:, j, dh * 512 : (dh + 1) * 512],
                                start=(j == 0),
                                stop=(j == 3),
                            )
                    ot = p_io.tile([P, D], F32, tag="io")
                    nc.scalar.activation(out=ot, in_=pso, func=ACT.Copy)
                    nc.sync.dma_start(out=dst_d[t * P : (t + 1) * P, :], in_=ot)

    _split_multi_waits(nc)
    return nc


def _get_program(key, used_chunks, cls, n_mixed):
    if key not in _prog_cache:
        _install_patches()
        _prog_cache[key] = _build_program(used_chunks, cls, n_mixed)
    return _prog_cache[key]


def kernel(
    input_tensor,
    ln_gamma,
    ln_beta,
    in_proj_w,
    in_proj_b,
    out_w,
    out_b,
    sequence_mask,
    attn_mask,
):
    x = np.asarray(input_tensor, np.float32)
    gamma = np.asarray(ln_gamma, np.float32)
    beta = np.asarray(ln_beta, np.float32)
    W = np.asarray(in_proj_w, np.float32)
    bias = np.asarray(in_proj_b, np.float32)
    Wo = np.asarray(out_w, np.float32)
    bo = np.asarray(out_b, np.float32)
    seqm = np.asarray(sequence_mask, bool)
    attn = np.asarray(attn_mask, bool)

    # ---- mask-derived program structure (identical across cores) ----
    used_chunks = [
        c for c in range(NT) if seqm[:, c * P : (c + 1) * P].any()
    ] or [0]
    attnT = attn.T
    cls, mixed = _classify_blocks(attnT)
    key = (tuple(used_chunks), tuple(tuple(r) for r in cls))
    nc = _get_program(key, used_chunks, cls, len(mixed))

    if mixed:
        mix_arr = np.stack(mixed, axis=0)
    else:
        mix_arr = np.zeros((1, P, 512), np.float32)

    # ---- host-side weight folding (gamma/beta/scale into W, b) ----
    scale_q = 1.0 / np.sqrt(HD)
    Wg = W * gamma[None, :]          # fold gamma
    bfold = bias + W @ beta          # fold beta
    in_maps = []
    for c in range(8):
        b = c // 2
        g = c % 2
        qs, ks, vs = 512 * g, D + 512 * g, 2 * D + 512 * g
        wq = Wg[qs : qs + 512] * scale_q
        wk = Wg[ks : ks + 512]
        wv = Wg[vs : vs + 512]
        bq = bfold[qs : qs + 512] * scale_q
        bk = bfold[ks : ks + 512]
        bv = bfold[vs : vs + 512]
        wqkT = np.ascontiguousarray(np.concatenate([wq, wk], axis=0).T)
        seqb = np.where(seqm[b], 0.0, NEG).astype(np.float32) - C_SHIFT
        in_maps.append(
            {
                "x": np.ascontiguousarray(x[b]),
                "xT": np.ascontiguousarray(x[b].T),
                "wqkT": wqkT,
                "wvT": np.ascontiguousarray(wv.T),
                "woT": np.ascontiguousarray(Wo[:, 512 * g : 512 * g + 512].T),
                "bqk": np.ascontiguousarray(np.concatenate([bq, bk])),
                "bv": np.ascontiguousarray(bv),
                "seqb": np.ascontiguousarray(seqb.reshape(NT, P).T),
                "mix": mix_arr,
            }
        )

    global _last_in_maps
    _last_in_maps = in_maps
    res = run_bass_kernel_spmd(nc, in_maps, list(range(8)))

    out = np.empty((2, B, T, D), np.float32)
    for b in range(B):
        r0, r1 = res.results[2 * b], res.results[2 * b + 1]
        out[0, b] = r0["out_on"] + r1["out_on"] + bo[None, :]
        out[1, b] = r0["out_off"] + r1["out_off"] + bo[None, :]
    return out



# revision 10
# speedup vs baseline: 1.0736x; 1.0736x over previous
"""Trainium2 Bass kernel for nn_ConformerMHSAV3 (LayerNorm + packed-QKV MHSA,
online/causal + offline/full-context variants, stacked output).

Sharding: 8 cores = 4 batches x 2 head-groups (8 heads each).  Each core
computes LN + its head-group's QKV + attention (both variants) + a partial
output projection; the host sums the two head-group partials per batch and
adds the output bias.

Key structure (v2, tuned for PE continuity + p-state ramp):
 - Inputs x/xT/Wqk/Wv stream as bf16 (half the DMA bytes; matmul rate is
   identical to fp32r at free>=256).  All on-chip tensors stay fp32/f32r.
 - LayerNorm is folded into the QKV matmuls: C/D run on RAW xT; the
   -mu*rstd correction + bias enter as a 9th accumulation matmul with
   lhsT=[w1; b] (host constants) and rhs=[munrstd; ones] (LN stats), and
   the rstd scale is applied in the PSUM->SBUF epilogue.  So the PE can
   start within ~2us of kernel start, overlapped with the LN stats.
 - C and D are ko-major (chunk-major) so each 256KB input chunk is
   consumed as soon as its DMA lands; 8 PSUM banks hold the 4 (C) / 8 (D)
   accumulators per pass.
 - Attention: softmax uses a constant shift (exact-softmax invariant) with
   key-padding folded into the exp bias; the causal/online variant reuses
   the offline exp for full blocks and multiplies a 0/1 mask for mixed
   blocks (no second exp).  Scores for chunk c+1 are emitted before PV of
   chunk c so the PE never waits on the scalar-engine exp.
 - The per-(head,variant) softmax division runs fully off the critical
   path: approx reciprocal (~5x faster than DVE reciprocal) + deep buffer
   pools so head h+1's compute never waits on head h's division chain.
"""

import numpy as np

import concourse.bass as bass
import concourse.mybir as mybir
import concourse.tile as tile
from concourse import mybir as _mybir
from concourse.bass_utils import run_bass_kernel_spmd
from concourse.vector_clock import ScopedClock, VectorClock

# ---------------------------------------------------------------------------
# Patches for this walrus build's 1-sync-wait-per-instruction cap.
# ---------------------------------------------------------------------------

_MAX_WAITS = 1


def _drain_and_barrier(self, tick_clock, wait_clock):
    gc = ScopedClock({None: tick_clock.global_clock})[None]
    n = len(gc)
    for p in [i for i in range(n) if gc[i] > 0]:
        nop = self.nc.sync.nop(nofuse=True, hint="tail_drain_split")
        partial = VectorClock([gc[j] if j == p else 0 for j in range(n)])
        wait_clock.add_sem_waits(nop.ins, ScopedClock({None: partial}))
    self.nc.sync.drain()
    self.nc.all_engine_barrier()
    assert self.sems is not None
    popped = self.nc._tile_sem_poison_stack.pop()
    assert popped is self._sem_poison
    self.nc.clear_and_free_semaphores(list(self.sems.allocated().values()))
    self.nc.all_engine_barrier()


def _install_patches():
    tile.TileContext._drain_and_barrier = _drain_and_barrier


def _split_multi_waits(nc):
    """Move all-but-one sem wait of each instruction onto same-engine NOPs
    inserted immediately before it (preserves per-engine program order)."""
    for f in nc.m.functions:
        for bb in f.blocks:
            insts = bb.instructions
            i = 0
            while i < len(insts):
                inst = insts[i]
                si = inst.sync_info
                if si is not None and si.on_wait and len(si.on_wait) > _MAX_WAITS:
                    extra = []
                    while len(si.on_wait) > _MAX_WAITS:
                        extra.append(si.on_wait.pop())
                    for w in extra:
                        nop = nc.engines[inst.engine].nop(nofuse=True).ins
                        for blk in f.blocks:
                            if blk.instructions and blk.instructions[-1] is nop:
                                blk.instructions.pop()
                                break
                        if nop.sync_info is None:
                            nop.sync_info = _mybir.SyncInfo(on_wait=[w], on_update=[])
                        else:
                            nop.sync_info.on_wait.append(w)
                        insts.insert(i, nop)
                        i += 1
                i += 1


# ---------------------------------------------------------------------------
# Problem constants (hardcoded per the self-contained-kernel contract).
# ---------------------------------------------------------------------------

B, T, D, H = 4, 1024, 1024, 16
HD = D // H          # 64
HL = H // 2          # 8 local heads per core
P = 128
NT = T // P          # 8 tiles of 128
EPS = 1e-5
C_SHIFT = 12.0       # constant softmax shift (exact-softmax invariant)
NEG = -1e30
F32 = mybir.dt.float32
F32R = mybir.dt.float32r
BF16 = mybir.dt.bfloat16

_prog_cache = {}


def _classify_blocks(attnT):
    """Per (k-chunk, q-half) classification of the online attention mask at
    PSUM-bank granularity (512 columns), so each bank hosts exactly one
    accumulation group (start=True clears has_written for the whole bank).
    Returns (cls[c][h] in {0:none,1:full,2+idx:masked}, 0/1 mult blocks)."""
    cls = [[0] * 2 for _ in range(NT)]
    mixed = []
    for c in range(NT):
        for h in range(2):
            blk = attnT[c * P : (c + 1) * P, h * 512 : (h + 1) * 512]
            if blk.all():
                cls[c][h] = 1
            elif not blk.any():
                cls[c][h] = 0
            else:
                cls[c][h] = 2 + len(mixed)
                mixed.append(np.where(blk, 1.0, 0.0).astype(np.float32))
    return cls, mixed


def _build_program(used_chunks, cls, n_mixed):
    """Build the SPMD Bass program.  Structure depends only on the masks'
    block classification, which is identical across cores."""
    nc = bass.Bass("TRN2", target_bir_lowering=False, debug=False)

    x_d = nc.declare_dram_parameter("x", [T, D], BF16, isOutput=False)
    xT_d = nc.declare_dram_parameter("xT", [D, T], BF16, isOutput=False)
    wqkT_d = nc.declare_dram_parameter("wqkT", [D, 2 * HL * HD], BF16, isOutput=False)
    wvT_d = nc.declare_dram_parameter("wvT", [D, HL * HD], BF16, isOutput=False)
    wb_d = nc.declare_dram_parameter("wb", [32, 2 * HL * HD], BF16, isOutput=False)
    vb_d = nc.declare_dram_parameter("vb", [32, HL * HD], BF16, isOutput=False)
    ones_d = nc.declare_dram_parameter("onesb", [1, T], BF16, isOutput=False)
    woT_d = nc.declare_dram_parameter("woT", [HL * HD, D], F32, isOutput=False)
    seqb_d = nc.declare_dram_parameter("seqb", [P, NT], F32, isOutput=False)
    nmx = max(n_mixed, 1)
    mix_d = nc.declare_dram_parameter("mix", [nmx, P, 512], F32, isOutput=False)
    oon_d = nc.declare_dram_parameter("out_on", [T, D], F32, isOutput=True)
    ooff_d = nc.declare_dram_parameter("out_off", [T, D], F32, isOutput=True)

    ACT = mybir.ActivationFunctionType
    OP = mybir.AluOpType

    first_off, last_off = used_chunks[0], used_chunks[-1]
    first_on = [None, None]
    last_on = [None, None]
    for qh in range(2):
        writers = [c for c in used_chunks if cls[c][qh] != 0]
        if writers:
            first_on[qh], last_on[qh] = writers[0], writers[-1]

    with tile.TileContext(nc) as tc:
        with (
            tc.tile_pool(name="pa", bufs=1) as p_a,      # xT -> oT_on
            tc.tile_pool(name="pb", bufs=1) as p_b,      # wqk -> oT_off
            tc.tile_pool(name="pq", bufs=1) as p_q,      # x(stats) -> qkT
            tc.tile_pool(name="pwo", bufs=1) as p_wo,    # wo
            tc.tile_pool(name="pw2", bufs=1) as p_w2,    # wv + mix
            tc.tile_pool(name="vv", bufs=1) as p_v,      # v
            tc.tile_pool(name="pp", bufs=3) as p_p,      # exp(P) tiles
            tc.tile_pool(name="blk", bufs=2) as p_blk,   # masked P blocks
            tc.tile_pool(name="ot", bufs=4) as p_ot,     # PSUM->SBUF o copies
            tc.tile_pool(name="rc", bufs=4) as p_rc,     # denom broadcast/recip
            tc.tile_pool(name="io", bufs=2) as p_io,     # F output staging
            tc.tile_pool(name="sm", bufs=1) as p_sm,     # small stats tiles
            tc.tile_pool(name="st", bufs=3) as p_st,     # transient small
            tc.tile_pool(name="dram", bufs=4, space="DRAM") as p_dram,
        ):
            # ---------------- upfront DMA issue --------------------------
            # sync ring: xT/wqk interleaved per 128-row chunk (C consumes
            # ko-major at ~2 matmul-us per chunk, DMA delivers at ~1.4us).
            xT_sb = p_a.tile([P, NT, T], BF16, tag="pa")
            wqk_sb = p_b.tile([P, NT, 2 * HL * HD], BF16, tag="pb")
            xT_v = xT_d[:].rearrange("(ko p) t -> p ko t", p=P)
            wqk_v = wqkT_d[:].rearrange("(ko p) m -> p ko m", p=P)
            for ko in range(NT):
                nc.sync.dma_start(out=xT_sb[:, ko, :], in_=xT_v[:, ko, :])
                nc.sync.dma_start(out=wqk_sb[:, ko, :], in_=wqk_v[:, ko, :])

            # scalar ring: x (stats), small weight rows, wv, seqb, mix, wo
            xa_sb = p_q.tile([P, NT, D], BF16, tag="pq")
            nc.scalar.dma_start(
                out=xa_sb, in_=x_d[:].rearrange("(n p) d -> p n d", p=P)
            )
            wb_sb = p_sm.tile([32, 2 * HL * HD], BF16, tag="wb")
            nc.scalar.dma_start(out=wb_sb, in_=wb_d[:])
            vb_sb = p_sm.tile([32, HL * HD], BF16, tag="vb")
            nc.scalar.dma_start(out=vb_sb, in_=vb_d[:])
            wv_sb = p_w2.tile([P, NT, HL * HD], BF16, tag="w2")
            nc.scalar.dma_start(
                out=wv_sb, in_=wvT_d[:].rearrange("(ko p) m -> p ko m", p=P)
            )
            seqb_sb = p_sm.tile([P, NT], F32, tag="seqb")
            nc.scalar.dma_start(out=seqb_sb, in_=seqb_d[:])
            mix_sb = p_w2.tile([P, nmx, 512], F32, tag="mix")
            nc.scalar.dma_start(out=mix_sb, in_=mix_d[:].rearrange("n p q -> p n q"))
            wo_sb = p_wo.tile([P, 4, D], F32R, tag="pwo")
            nc.scalar.dma_start(
                out=wo_sb, in_=woT_d[:].rearrange("(j p) m -> p j m", p=P).bitcast(F32R)
            )

            # ---------------- Phase A: LN statistics ---------------------
            mu_all = p_sm.tile([P, NT], F32, tag="mu")
            rstd_all = p_sm.tile([P, NT], F32, tag="rstd")
            munbf = p_sm.tile([P, NT], BF16, tag="munbf")
            eps_t = p_sm.tile([P, 1], F32, tag="eps")
            nc.vector.memset(eps_t, EPS)

            for t in range(NT):
                stats = p_st.tile([P, 2, 6], F32, tag="bst")
                xv = xa_sb[:, t, :].rearrange("p (s q) -> p s q", s=2)
                for s in range(2):
                    nc.vector.bn_stats(out=stats[:, s, :], in_=xv[:, s, :])
                mv = p_st.tile([P, 2], F32, tag="mv")
                nc.vector.bn_aggr(out=mv, in_=stats)
                nc.gpsimd.tensor_copy(out=mu_all[:, t : t + 1], in_=mv[:, 0:1])
                # sqrt(var + eps); reciprocal after the loop
                nc.scalar.activation(
                    out=rstd_all[:, t : t + 1], in_=mv[:, 1:2],
                    func=ACT.Sqrt, bias=eps_t, scale=1.0,
                )
            nc.vector.reciprocal(out=rstd_all, in_=rstd_all)
            muxr = p_st.tile([P, NT], F32, tag="muxr")
            nc.vector.tensor_tensor(out=muxr, in0=mu_all, in1=rstd_all, op=OP.mult)
            nc.vector.tensor_scalar(
                out=munbf, in0=muxr, scalar1=-1.0, scalar2=None, op0=OP.mult
            )
            # prefetch the exp ACT table during C (A used the sqrt set)
            dum = p_st.tile([P, 1], F32, tag="dum")
            nc.scalar.activation(out=dum, in_=eps_t, func=ACT.Exp)

            # stripe [P, NT] -> DRAM rows (t = 128*tile + p ordering)
            scrR = p_dram.tile([T], F32, tag="scrR")
            nc.sync.dma_start(out=scrR[:].rearrange("(n p) -> p n", p=P), in_=rstd_all)
            scrM = p_dram.tile([T], BF16, tag="scrM")
            nc.sync.dma_start(out=scrM[:].rearrange("(n p) -> p n", p=P), in_=munbf)
            # broadcast rstd across all 128 partitions (for the C epilogue)
            rstd_bc = p_sm.tile([P, T], F32, tag="bc")
            nc.sync.dma_start(
                out=rstd_bc,
                in_=bass.AP(tensor=scrR.tensor, offset=scrR.offset, ap=[[0, P], [1, T]]),
            )
            # [ones; munrstd; zero-pad] rows for the bias matmuls
            # (32 partitions: the PE rejects 2-partition operands)
            br_sb = p_sm.tile([32, T], BF16, tag="br")
            nc.vector.memset(br_sb, 0.0)
            nc.sync.dma_start(out=br_sb[0:1, :], in_=ones_d[:])
            nc.sync.dma_start(out=br_sb[1:2, :], in_=scrM[None, :])

            # ---------------- Phase C: qkT = Wqk' @ x^T + LN-fold --------
            # ko-major over input chunks, two passes of 4 m-tiles
            # (4 x [P,T] fp32 accumulators = all 8 PSUM banks).
            qkT_sb = p_q.tile([P, NT, T], F32R, tag="pq")
            with tc.tile_pool(name="ps4", bufs=4, space="PSUM") as p_ps4:
                for half in range(2):
                    mts = list(range(half * 4, half * 4 + 4))
                    psqs = []
                    for i in range(4):
                        psq = p_ps4.tile([P, T], F32, tag="ps4")
                        psqs.append(psq)
                    for ko in range(NT):
                        for i, mt in enumerate(mts):
                            for qh in range(2):
                                nc.tensor.matmul(
                                    psqs[i][:, qh * 512 : (qh + 1) * 512],
                                    lhsT=wqk_sb[:, ko, mt * P : (mt + 1) * P],
                                    rhs=xT_sb[:, ko, qh * 512 : (qh + 1) * 512],
                                    start=(ko == 0),
                                    stop=False,
                                )
                    for i, mt in enumerate(mts):
                        for qh in range(2):
                            nc.tensor.matmul(
                                psqs[i][:, qh * 512 : (qh + 1) * 512],
                                lhsT=wb_sb[:, mt * P : (mt + 1) * P],
                                rhs=br_sb[:, qh * 512 : (qh + 1) * 512],
                                start=False,
                                stop=True,
                            )
                        nc.vector.tensor_tensor(
                            out=qkT_sb[:, mt, :], in0=psqs[i], in1=rstd_bc, op=OP.mult
                        )

                # ------------ Phase D: v = x^T' @ Wv + LN-fold -----------
                # ko-major, 8 [P,512] accumulators = 8 banks (bank-aligned
                # halves of 4 pool tiles).
                v_sb = p_v.tile([P, NT, HL * 65], F32R, tag="vv")
                nc.vector.memset(
                    v_sb.rearrange("p c (h j) -> p c h j", j=65)[:, :, :, 64]
                    .bitcast(F32),
                    1.0,
                )
                psvs = []
                for i in range(4):
                    psv = p_ps4.tile([P, T], F32, tag="ps4")
                    psvs.append(psv)

                def psv_sl(t):
                    return psvs[t // 2][:, (t % 2) * 512 : (t % 2) * 512 + 512]

                for ko in range(NT):
                    for t in range(NT):
                        nc.tensor.matmul(
                            psv_sl(t),
                            lhsT=xT_sb[:, ko, t * P : (t + 1) * P],
                            rhs=wv_sb[:, ko, :],
                            start=(ko == 0),
                            stop=False,
                        )
                for t in range(NT):
                    nc.tensor.matmul(
                        psv_sl(t),
                        lhsT=br_sb[:, t * P : (t + 1) * P],
                        rhs=vb_sb[:],
                        start=False,
                        stop=True,
                    )
                    nc.vector.tensor_scalar(
                        out=v_sb.rearrange("p c (h j) -> p c h j", j=65)[
                            :, t, :, 0:64
                        ],
                        in0=psv_sl(t).rearrange("p (h j) -> p h j", j=HD),
                        scalar1=rstd_all[:, t : t + 1],
                        scalar2=None,
                        op0=OP.mult,
                    )

            # ---------------- Phase E: attention per head ----------------
            oT_on = p_a.tile([P, 4, T], F32R, tag="pa")
            oT_off = p_b.tile([P, 4, T], F32R, tag="pb")

            with (
                tc.tile_pool(name="pss", bufs=2, space="PSUM") as p_pss,
                tc.tile_pool(name="po", bufs=2, space="PSUM") as p_po,
            ):
                for h in range(HL):
                    par = h % 2
                    base = 64 * par
                    group = h // 2
                    qT_h = qkT_sb[base : base + 64, group, :]
                    kT_h = qkT_sb[base : base + 64, 4 + group, :]
                    vlo = 65 * h
                    pon_t = p_po.tile([P, T], F32, tag="po")
                    poff_t = p_po.tile([P, T], F32, tag="po")
                    pon = pon_t[0:65]
                    poff = poff_t[0:65]

                    pss_l = {}

                    def emit_scores(ci, c):
                        pss = p_pss.tile([P, T], F32, tag="pss")
                        pss_l[ci] = pss
                        for qh in range(2):
                            nc.tensor.matmul(
                                pss[:, qh * 512 : (qh + 1) * 512],
                                lhsT=kT_h[:, c * P : (c + 1) * P],
                                rhs=qT_h[:, qh * 512 : (qh + 1) * 512],
                                start=True,
                                stop=True,
                            )

                    def emit_pv(ci, c):
                        pss = pss_l.pop(ci)
                        pofc = p_p.tile([P, T], F32R, tag="pp")
                        nc.scalar.activation(
                            out=pofc, in_=pss, func=ACT.Exp,
                            bias=seqb_sb[:, c : c + 1], scale=1.0,
                        )
                        lhsT = v_sb[:, c, vlo : vlo + 65]
                        blks = {}
                        for qh in range(2):
                            k = cls[c][qh]
                            if k >= 2:
                                pblk = p_blk.tile([P, 512], F32R, tag="blk")
                                nc.vector.tensor_tensor(
                                    out=pblk,
                                    in0=pofc[:, qh * 512 : (qh + 1) * 512].bitcast(F32),
                                    in1=mix_sb[:, k - 2, :],
                                    op=OP.mult,
                                )
                                blks[qh] = pblk
                        for qh in range(2):
                            nc.tensor.matmul(
                                poff[:, qh * 512 : (qh + 1) * 512],
                                lhsT=lhsT,
                                rhs=pofc[:, qh * 512 : (qh + 1) * 512],
                                start=(c == first_off),
                                stop=(c == last_off),
                            )
                        for qh in range(2):
                            k = cls[c][qh]
                            if k == 0 or first_on[qh] is None:
                                continue
                            qsl = slice(qh * 512, (qh + 1) * 512)
                            rhs = pofc[:, qsl] if k == 1 else blks[qh]
                            nc.tensor.matmul(
                                pon[:, qsl],
                                lhsT=lhsT,
                                rhs=rhs,
                                start=(c == first_on[qh]),
                                stop=(c == last_on[qh]),
                            )

                    # software-pipelined: scores(c+1) issue before PV(c)
                    emit_scores(0, used_chunks[0])
                    for ci in range(1, len(used_chunks)):
                        emit_scores(ci, used_chunks[ci])
                        emit_pv(ci - 1, used_chunks[ci - 1])
                    emit_pv(len(used_chunks) - 1, used_chunks[-1])

                    # softmax division, fully off the critical path
                    dlo = base
                    for pt, dst in ((pon_t, oT_on), (poff_t, oT_off)):
                        otmp = p_ot.tile([P, T], F32, tag="ot")
                        nc.scalar.activation(
                            out=otmp[0:65, :], in_=pt[0:65], func=ACT.Copy
                        )
                        # narrow reciprocal: stripe the sums row across the
                        # 128 partitions (8 elem/partition) so the slow DVE
                        # reciprocal costs ~0.1us instead of 6us, then
                        # broadcast the inverted row.
                        drow = p_dram.tile([T], F32, tag="drow")
                        nc.sync.dma_start(out=drow[None, :], in_=otmp[64:65, :])
                        rsm = p_st.tile([P, NT], F32, tag="rsm", bufs=4)
                        nc.sync.dma_start(
                            out=rsm, in_=drow[:].rearrange("(n p) -> p n", p=P)
                        )
                        nc.vector.reciprocal(out=rsm, in_=rsm)
                        drow2 = p_dram.tile([T], F32, tag="drow2")
                        nc.sync.dma_start(
                            out=drow2[:].rearrange("(n p) -> p n", p=P), in_=rsm
                        )
                        rct = p_rc.tile([P, T], F32, tag="rc")
                        nc.sync.dma_start(
                            out=rct[0:64, :],
                            in_=bass.AP(
                                tensor=drow2.tensor,
                                offset=drow2.offset,
                                ap=[[0, 64], [1, T]],
                            ),
                        )
                        nc.vector.tensor_tensor(
                            out=dst[dlo : dlo + 64, group, :],
                            in0=otmp[0:64],
                            in1=rct[0:64, :],
                            op=OP.mult,
                        )
                    # zero any online q-halves no chunk wrote (fully masked)
                    for qh in range(2):
                        if first_on[qh] is None:
                            nc.vector.memset(
                                oT_on[dlo : dlo + 64, group, qh * 512 : (qh + 1) * 512]
                                .bitcast(F32),
                                0.0,
                            )

            # ---------------- Phase F: output projection -----------------
            with tc.tile_pool(name="pso", bufs=2, space="PSUM") as p_pso:
                for src, dst_d in ((oT_on, oon_d), (oT_off, ooff_d)):
                    for t in range(NT):
                        pso = p_pso.tile([P, T], F32, tag="pso")
                        for dh in range(2):
                            for j in range(4):
                                nc.tensor.matmul(
                                    pso[:, dh * 512 : (dh + 1) * 512],
                                    lhsT=src[:, j, t * P : (t + 1) * P],
                                    rhs=wo_sb[:, j, dh * 512 : (dh + 1) * 512],
                                    start=(j == 0),
                                    stop=(j == 3),
                                )
                        ot = p_io.tile([P, D], F32, tag="io")
                        nc.scalar.activation(out=ot, in_=pso, func=ACT.Copy)
                        nc.sync.dma_start(out=dst_d[t * P : (t + 1) * P, :], in_=ot)

    _split_multi_waits(nc)
    return nc


def _get_program(key, used_chunks, cls, n_mixed):
    if key not in _prog_cache:
        _install_patches()
        _prog_cache[key] = _build_program(used_chunks, cls, n_mixed)
    return _prog_cache[key]


def kernel(
    input_tensor,
    ln_gamma,
    ln_beta,
    in_proj_w,
    in_proj_b,
    out_w,
    out_b,
    sequence_mask,
    attn_mask,
):
    import ml_dtypes

    BF = ml_dtypes.bfloat16

    x = np.asarray(input_tensor, np.float32)
    gamma = np.asarray(ln_gamma, np.float32)
    beta = np.asarray(ln_beta, np.float32)
    W = np.asarray(in_proj_w, np.float32)
    bias = np.asarray(in_proj_b, np.float32)
    Wo = np.asarray(out_w, np.float32)
    bo = np.asarray(out_b, np.float32)
    seqm = np.asarray(sequence_mask, bool)
    attn = np.asarray(attn_mask, bool)

    # ---- mask-derived program structure (identical across cores) ----
    used_chunks = [
        c for c in range(NT) if seqm[:, c * P : (c + 1) * P].any()
    ] or [0]
    attnT = attn.T
    cls, mixed = _classify_blocks(attnT)
    key = (tuple(used_chunks), tuple(tuple(r) for r in cls))
    nc = _get_program(key, used_chunks, cls, len(mixed))

    if mixed:
        mix_arr = np.stack(mixed, axis=0)
    else:
        mix_arr = np.zeros((1, P, 512), np.float32)

    # ---- host-side weight folding (gamma/beta/scale into W, b) ----
    scale_q = 1.0 / np.sqrt(HD)
    Wg = W * gamma[None, :]          # fold gamma
    bfold = bias + W @ beta          # fold beta
    in_maps = []
    for c in range(8):
        b = c // 2
        g = c % 2
        qs, ks, vs = 512 * g, D + 512 * g, 2 * D + 512 * g
        wq = Wg[qs : qs + 512] * scale_q
        wk = Wg[ks : ks + 512]
        wv = Wg[vs : vs + 512]
        bq = bfold[qs : qs + 512] * scale_q
        bk = bfold[ks : ks + 512]
        bv = bfold[vs : vs + 512]
        wqkT = np.ascontiguousarray(np.concatenate([wq, wk], axis=0).T)
        wvT = np.ascontiguousarray(wv.T)
        # rows pair with br=[ones; munrstd]: row0 bias, row1 weight-rowsum
        wb = np.zeros((32, 2 * HL * HD), np.float32)
        wb[0] = np.concatenate([bq, bk])
        wb[1] = wqkT.sum(axis=0)
        vb = np.zeros((32, HL * HD), np.float32)
        vb[0] = bv
        vb[1] = wvT.sum(axis=0)
        seqb = np.where(seqm[b], 0.0, NEG).astype(np.float32) - C_SHIFT
        in_maps.append(
            {
                "x": np.ascontiguousarray(x[b]).astype(BF),
                "xT": np.ascontiguousarray(x[b].T).astype(BF),
                "wqkT": wqkT.astype(BF),
                "wvT": wvT.astype(BF),
                "wb": np.ascontiguousarray(wb).astype(BF),
                "vb": np.ascontiguousarray(vb).astype(BF),
                "onesb": np.ones((1, T), dtype=BF),
                "woT": np.ascontiguousarray(Wo[:, 512 * g : 512 * g + 512].T),
                "seqb": np.ascontiguousarray(seqb.reshape(NT, P).T),
                "mix": mix_arr,
            }
        )

    global _last_in_maps
    _last_in_maps = in_maps
    res = run_bass_kernel_spmd(nc, in_maps, list(range(8)))

    out = np.empty((2, B, T, D), np.float32)
    for b in range(B):
        r0, r1 = res.results[2 * b], res.results[2 * b + 1]
        out[0, b] = r0["out_on"] + r1["out_on"] + bo[None, :]
        out[1, b] = r0["out_off"] + r1["out_off"] + bo[None, :]
    return out
